# revision 8
# baseline (speedup 1.0000x reference)
"""Trainium2 Bass kernel for sliding-window causal attention block.

Reference computation (per batch b):
  qh = (q @ wq.T)  -> [S, H, Dh], RoPE'd; kh likewise; vh = v @ wv.T
  scores = qh . kh / sqrt(Dh), sliding-window causal (j in (i-512, i])
  out = softmax(scores) @ vh  -> [S, H*Dh] @ wo.T -> [S, D]

Sharding: 8 cores = 2 batches x 4 head-groups (4 heads each).
Each core computes y_part[b] = attn(heads g) @ wo[:, g].T  (f16 partial);
host sums the 4 partials per batch and casts to f16.

Key optimizations over the f16 baseline (CoreSim makespan 126.6us):
  - Projections in compensated fp8 with DoubleRow perf mode (0.5 cyc/col):
    x = x_hi(e4m3) + x_lo(e5m2), w*32 = w_hi(e4m3) + w_lo(e5m2), both split
    on the host; on-chip x@w = xh@wh + xl@wh + xh@wl (lo x lo dropped).
    12 DoubleRow matmuls replace 8 f16 matmuls per K=1024 contraction:
    0.75x PE cost at ~0.2% rms error (measured rel-err 0.0016 vs 2e-2 gate).
    The *32 weight scale keeps w out of e4m3's subnormal range; it is
    folded out via exp scale 2^-13 (q,k) and woT/32 (v path).
  - Window masks: no PE mask transposes into PSUM; instead exp runs
    unmasked and the 1-2 boundary chunks of each (tile, head) are zeroed
    post-exp with in-place binary-mask multiplies on the idle GpSimd
    engine (PE -12us).
  - Phase interleaving: attention for q-tiles 4(sc-1)..4sc-1 is emitted
    between projection s-chunks sc and sc+1, so PE never drains while
    ACT/DVE catch up, and the input DMAs overlap compute.
  - DMA consolidation: each x tensor chunk loads with a single 3D-AP DMA
    ([128, 8, 512] from the [D, S] DRAM view) instead of 8 per-kc DMAs;
    y stores issue from the Pool queue to unload SP.
  - Engine balance (v1 cost model): PE ~72us, DVE ~56, ACT ~53, Pool ~38,
    SP ~47.  RoPE runs as ACT copy + PE perm-matmul + DVE t2-mul +
    Pool t1-mul + Pool add; v copies and y casts on DVE; q/k copies on ACT.
"""

import os
import sys

import numpy as np

for _p in ("/opt/trn_rl_repo", "/root/.axon_site/_ro/trn_rl_repo"):
    if os.path.isdir(_p) and _p not in sys.path:
        sys.path.insert(0, _p)

import ml_dtypes

DIM = 1024
NUM_HEADS = 16
HEAD_DIM = 64
WINDOW = 512
S = 2048
B = 2
HPC = 4  # heads per core
E = HPC * HEAD_DIM  # 256 = per-core hidden slice
N_CORES = 8
ST = S // 128  # 16 query tiles of 128
KC = DIM // 128  # 8 contraction chunks for projections
WS = 32.0  # weight pre-scale (keeps w out of e4m3 subnormals)
EXP_SCALE = 0.125 / (WS * WS)  # 2^-13, folds away q,k scales

E4 = ml_dtypes.float8_e4m3
E5 = ml_dtypes.float8_e5m2


def _rope_tables():
    # A/B factor tables in the RoPE-permuted [p, s] layout, f32.
    f = np.arange(32, dtype=np.float64)
    inv_freq = 1.0 / (10000.0 ** (2.0 * f / HEAD_DIM))  # [32]
    ang = np.arange(S, dtype=np.float64)[None, :] * inv_freq[:, None]  # [32, S]
    cos = np.cos(ang)
    sin = np.sin(ang)
    A = np.empty((128, S), dtype=np.float32)
    Bt = np.empty((128, S), dtype=np.float32)
    for blk in range(2):  # two 64-partition head blocks per tile
        o = blk * 64
        A[o : o + 32] = cos
        A[o + 32 : o + 64] = cos
        Bt[o : o + 32] = -sin
        Bt[o + 32 : o + 64] = sin
    return A, Bt


def _consts():
    A, Bt = _rope_tables()
    # binary masks in the transposed-scores [jj, ii] orientation
    jj = np.arange(128)[:, None]
    ii = np.arange(128)[None, :]
    mbinD = (jj <= ii).astype(np.float16)  # diagonal chunk: key j <= query i
    mbinL = (jj > ii).astype(np.float16)  # leftmost chunk: j > i - WINDOW
    permM = np.zeros((128, 128), dtype=np.float16)
    for m in range(128):
        partner = m + 32 if (m % 64) < 32 else m - 32
        permM[partner, m] = 1.0
    ident = np.eye(128, dtype=np.float16)
    return {
        "ropeA": A.astype(np.float16),
        "ropeB": Bt.astype(np.float16),
        "mbinD": mbinD,
        "mbinL": mbinL,
        "permM": permM,
        "ident": ident,
    }


def _head_perm():
    # within each head: evens then odds
    p = np.empty(E, dtype=np.int64)
    for h in range(HPC):
        base = h * HEAD_DIM
        p[base : base + 32] = base + np.arange(0, 64, 2)
        p[base + 32 : base + 64] = base + np.arange(1, 64, 2)
    return p


def _split8(x64):
    """hi = e4m3(x), lo = e5m2(x - hi); x64 must be float32/64."""
    hi = np.asarray(x64).astype(E4)
    lo = (np.asarray(x64, dtype=np.float32) - hi.astype(np.float32)).astype(E5)
    return hi, lo


def build_bass(do_compile=True):
    import concourse.bacc as bacc
    import concourse.mybir as mybir
    import concourse.tile as tile

    f16 = mybir.dt.float16
    f32 = mybir.dt.float32
    e4 = mybir.dt.float8e4
    e5 = mybir.dt.float8e5
    DR = mybir.MatmulPerfMode.DoubleRow
    Exp = mybir.ActivationFunctionType.Exp

    nc = bacc.Bacc("TRN2")

    xin = {}
    for t in ("q", "k", "v"):
        for p, dt in (("h", e4), ("l", e5)):
            xin[t + p] = nc.dram_tensor(f"x{t}{p}", [DIM, S], dt, kind="ExternalInput")
    win = {}
    for t in ("q", "k", "v"):
        for p, dt in (("h", e4), ("l", e5)):
            win[t + p] = nc.dram_tensor(f"w{t}{p}", [DIM, E], dt, kind="ExternalInput")
    woT = nc.dram_tensor("woT", [E, DIM], f16, kind="ExternalInput")
    ropeA = nc.dram_tensor("ropeA", [128, S], f16, kind="ExternalInput")
    ropeB = nc.dram_tensor("ropeB", [128, S], f16, kind="ExternalInput")
    mbinD = nc.dram_tensor("mbinD", [128, 128], f16, kind="ExternalInput")
    mbinL = nc.dram_tensor("mbinL", [128, 128], f16, kind="ExternalInput")
    permM = nc.dram_tensor("permM", [128, 128], f16, kind="ExternalInput")
    ident = nc.dram_tensor("ident", [128, 128], f16, kind="ExternalInput")
    y = nc.dram_tensor("y", [S, DIM], f16, kind="ExternalOutput")

    with tile.TileContext(nc) as tc:
        # All pools stay open for the whole kernel (no recycled-slot release
        # fan-in; see baseline notes on DVE sync-wait limits).
        with tc.tile_pool(name="res", bufs=1) as res, \
             tc.tile_pool(name="xp", bufs=2) as xp, \
             tc.tile_pool(name="rawp", bufs=2) as rawp, \
             tc.tile_pool(name="t12", bufs=2) as t12, \
             tc.tile_pool(name="ptp", bufs=4) as ptp, \
             tc.tile_pool(name="sb2", bufs=2) as sb2, \
             tc.tile_pool(name="yb", bufs=3) as yb, \
             tc.tile_pool(name="pu", bufs=3, space="PSUM") as pu, \
             tc.tile_pool(name="pst", bufs=2, space="PSUM") as stp, \
             tc.tile_pool(name="pop", bufs=1, space="PSUM") as op:
            # resident tensors
            qT = res.tile([128, 2, S], f16)
            kT = res.tile([128, 2, S], f16)
            v_sb = res.tile([128, ST, HPC, 65], f16)
            woT_sb = res.tile([128, 2, DIM], f16)
            mbinD_sb = res.tile([128, 128], f16)
            mbinL_sb = res.tile([128, 128], f16)
            ident_sb = res.tile([128, 128], f16)
            A_sb = res.tile([128, S], f16)
            B_sb = res.tile([128, S], f16)
            perm_sb = res.tile([128, 128], f16)
            w_sb = {}
            for t in ("q", "k", "v"):
                for p, dt in (("h", e4), ("l", e5)):
                    w_sb[t + p] = res.tile([128, KC, E], dt, name=f"w_{t}{p}")

            def load_x1(sc, t, eng=None):
                eng = eng or nc.sync
                ssl = slice(sc * 512, (sc + 1) * 512)
                out = {}
                for p, dt in (("h", e4), ("l", e5)):
                    xt = xp.tile([128, KC, 512], dt, tag=f"x{t}{p}", name=f"x{t}{p}_t")
                    eng.dma_start(
                        out=xt,
                        in_=xin[t + p][:, ssl].rearrange("(c p) s -> p c s", p=128),
                    )
                    out[t + p] = xt
                return out

            # Startup: wq + xq0 stream on SP while xk0/xv0 load in parallel on
            # the (idle) ACT and DVE queues; all other consts go on the Pool
            # queue.  This gets the first projection running ~4us in.
            xts = {}
            nc.sync.dma_start(
                out=w_sb["qh"], in_=win["qh"][:].rearrange("(c p) e -> p c e", p=128)
            )
            xts.update(load_x1(0, "q"))
            xts.update(load_x1(0, "k", nc.scalar))
            xts.update(load_x1(0, "v", nc.gpsimd))
            nc.sync.dma_start(
                out=w_sb["ql"], in_=win["ql"][:].rearrange("(c p) e -> p c e", p=128)
            )
            for t in ("k", "v"):
                for p in ("h", "l"):
                    nc.sync.dma_start(
                        out=w_sb[t + p],
                        in_=win[t + p][:].rearrange("(c p) e -> p c e", p=128),
                    )
            nc.gpsimd.dma_start(out=A_sb, in_=ropeA[:])
            nc.gpsimd.dma_start(out=B_sb, in_=ropeB[:])
            nc.gpsimd.dma_start(out=perm_sb, in_=permM[:])
            nc.gpsimd.dma_start(out=mbinD_sb, in_=mbinD[:])
            nc.gpsimd.dma_start(out=mbinL_sb, in_=mbinL[:])
            nc.gpsimd.dma_start(out=ident_sb, in_=ident[:])
            nc.gpsimd.dma_start(
                out=woT_sb, in_=woT[:].rearrange("(c p) n -> p c n", p=128)
            )
            nc.any.memset(v_sb[:, :, :, 64:65], 1.0)

            def load_x(sc):
                out = {}
                for t in ("q", "k", "v"):
                    out.update(load_x1(sc, t))
                return out

            def proj_dr(ps_out, lhs_h, lhs_l, rhs_h, rhs_l, lhsl, rhsl):
                # 12 DoubleRow matmuls: hi@hi + lo@hi + hi@lo over 4 kc-pairs
                first = True
                for wh, xh in ((lhs_h, rhs_h), (lhs_l, rhs_h), (lhs_h, rhs_l)):
                    for c in range(4):
                        cs = slice(2 * c, 2 * c + 2)
                        nc.tensor.matmul(
                            ps_out,
                            lhsT=wh[:, cs, lhsl],
                            rhs=xh[:, cs, rhsl],
                            start=first,
                            stop=(wh is lhs_h and xh is rhs_l and c == 3),
                            perf_mode=DR,
                        )
                        first = False

            def attn_tile(t):
                c0 = max(0, t - 4)
                ncv = t - c0 + 1
                tsl = slice(t * 128, (t + 1) * 128)
                # one PSUM bank holds the PV accumulator (floats 0:264) and
                # the two attnT transpose scratch regions (f16 bitcast views)
                pot = op.tile([128, 512], f32, tag="po")
                po = pot[:, 0:264].rearrange("p (h d) -> p h d", h=HPC)
                for h in range(HPC):
                    ec, hh = h // 2, h % 2
                    psl = slice(hh * 64, (hh + 1) * 64)
                    pst = stp.tile([128, 5, 128], f32, tag="st")
                    for si in range(ncv):
                        c = c0 + si
                        nc.tensor.matmul(
                            pst[:, si, :],
                            lhsT=kT[psl, ec, c * 128 : (c + 1) * 128],
                            rhs=qT[psl, ec, tsl],
                            start=True,
                            stop=True,
                        )
                    pt = ptp.tile([128, 5, 128], f16, tag="pt")
                    nc.scalar.activation(
                        pt[:, 0:ncv, :], pst[:, 0:ncv, :], Exp, scale=EXP_SCALE
                    )
                    # zero invalid boundary entries post-exp (GpSimd, SBUF-only)
                    if t >= 4:
                        nc.gpsimd.tensor_mul(pt[:, 0, :], pt[:, 0, :], mbinL_sb)
                    nc.gpsimd.tensor_mul(
                        pt[:, ncv - 1, :], pt[:, ncv - 1, :], mbinD_sb
                    )
                    for si in range(ncv):
                        c = c0 + si
                        nc.tensor.matmul(
                            po[:, h, 0:65],
                            lhsT=pt[:, si, :],
                            rhs=v_sb[:, c, h, :],
                            start=(si == 0),
                            stop=(si == ncv - 1),
                        )
                rc = sb2.tile([128, HPC, 1], f32, tag="rc")
                nc.vector.reciprocal(rc, po[:, :, 64:65])
                attn_t = sb2.tile([128, HPC, 64], f16, tag="attn")
                nc.vector.tensor_mul(
                    attn_t, po[:, :, 0:64], rc.broadcast_to([128, HPC, 64])
                )
                attnT_t = sb2.tile([128, 2, 128], f16, tag="attnT", bufs=3)
                attn_flat = attn_t.rearrange("p h d -> p (h d)")
                for ec in range(2):
                    ptr = pot[:, 264 + 64 * ec : 328 + 64 * ec].bitcast(f16)
                    nc.tensor.transpose(
                        ptr, attn_flat[:, ec * 128 : (ec + 1) * 128], ident_sb
                    )
                    nc.vector.tensor_copy(attnT_t[:, ec, :], ptr)
                return attnT_t

            def y_emit(t, attnT_t, tail=False):
                tsl = slice(t * 128, (t + 1) * 128)
                y_sb = yb.tile([128, DIM], f16, tag="ysb")
                for nch in range(2):
                    nsl = slice(nch * 512, (nch + 1) * 512)
                    py = pu.tile([128, 512], f32, tag="pu", name="py")
                    for ec in range(2):
                        nc.tensor.matmul(
                            py,
                            lhsT=attnT_t[:, ec, :],
                            rhs=woT_sb[:, ec, nsl],
                            start=(ec == 0),
                            stop=(ec == 1),
                        )
                    nc.vector.tensor_copy(y_sb[:, nsl], py)
                    if tail:
                        nc.sync.dma_start(out=y[tsl, nsl], in_=y_sb[:, nsl])
                if not tail:
                    nc.gpsimd.dma_start(out=y[tsl, :], in_=y_sb)

            pend = []

            def run_tile(t):
                attnT_t = attn_tile(t)
                pend.append((t, attnT_t))
                if len(pend) > 1:
                    y_emit(*pend.pop(0))

            def qk_unit(xts, sc, t, ec, out_sb, raw):
                ssl = slice(sc * 512, (sc + 1) * 512)
                esl = slice(ec * 128, (ec + 1) * 128)
                ps = pu.tile([128, 512], f32, tag="pu", name="ps_qk")
                proj_dr(
                    ps, w_sb[t + "h"], w_sb[t + "l"],
                    xts[t + "h"], xts[t + "l"], esl, slice(None),
                )
                nc.scalar.copy(raw[:, ec, :], ps)
                psh = pu.tile([128, 512], f32, tag="pu", name="psh")
                nc.tensor.matmul(
                    psh, lhsT=perm_sb, rhs=raw[:, ec, :],
                    start=True, stop=True,
                )
                t2 = t12.tile([128, 512], f16, tag="t2")
                nc.vector.tensor_mul(t2, psh, B_sb[:, ssl])
                t1 = t12.tile([128, 512], f16, tag="t1")
                nc.gpsimd.tensor_mul(t1, raw[:, ec, :], A_sb[:, ssl])
                nc.gpsimd.tensor_add(out_sb[:, ec, ssl], t1, t2)

            def v_unit(xts, sc, st4):
                sc16 = sc * 4 + st4
                stsl = slice(st4 * 128, (st4 + 1) * 128)
                ps = pu.tile([128, 512], f32, tag="pu", name="ps_v")
                proj_dr(
                    ps[:, 0:E], xts["vh"], xts["vl"],
                    w_sb["vh"], w_sb["vl"], stsl, slice(None),
                )
                nc.vector.tensor_copy(
                    v_sb[:, sc16, :, 0:64],
                    ps[:, 0:E].rearrange("p (h d) -> p h d", h=HPC),
                )

            def proj_units(xts, sc):
                raws = {
                    "q": rawp.tile([128, 2, 512], f16, tag="rawq", name="rawq"),
                    "k": rawp.tile([128, 2, 512], f16, tag="rawk", name="rawk"),
                }
                units = []
                for t, out_sb in (("q", qT), ("k", kT)):
                    for ec in range(2):
                        units.append(
                            lambda t=t, ec=ec, o=out_sb: qk_unit(
                                xts, sc, t, ec, o, raws[t]
                            )
                        )
                for st4 in range(4):
                    units.append(lambda st4=st4: v_unit(xts, sc, st4))
                return units

            for sc in range(4):
                xts_next = load_x(sc + 1) if sc + 1 < 4 else None
                units = proj_units(xts, sc)
                if sc == 0:
                    for u in units:
                        u()
                else:
                    # interleave: attn tile of the previous chunk between
                    # pairs of projection units so PE never drains
                    for i in range(4):
                        run_tile(4 * (sc - 1) + i)
                        units[2 * i]()
                        units[2 * i + 1]()
                xts = xts_next
            for t in range(12, ST):
                run_tile(t)
            while pend:
                t_, a_ = pend.pop(0)
                y_emit(t_, a_, tail=not pend)
    if do_compile:
        nc.compile()
    return nc


_CACHE = {}


def _get_nc():
    if "nc" not in _CACHE:
        _CACHE["nc"] = build_bass()
    return _CACHE["nc"]


def _in_maps(q, k, v, wq, wk, wv, wo):
    consts = _consts()
    perm = _head_perm()
    # per-batch x splits, shared by the 4 head-group cores of that batch
    xsplit = []
    for b in range(B):
        m = {}
        for nm, x in (("q", q), ("k", k), ("v", v)):
            hi, lo = _split8(np.ascontiguousarray(x[b].T, dtype=np.float32))
            m[f"x{nm}h"] = hi
            m[f"x{nm}l"] = lo
        xsplit.append(m)
    maps = []
    for c in range(N_CORES):
        b, g = c // 4, c % 4
        esl = slice(g * E, (g + 1) * E)
        m = dict(xsplit[b])
        for nm, w, permute in (("q", wq, True), ("k", wk, True), ("v", wv, False)):
            w_c = w[esl]
            if permute:
                w_c = w_c[perm]
            wt = np.ascontiguousarray(w_c.T, dtype=np.float32) * np.float32(WS)
            hi, lo = _split8(wt)
            m[f"w{nm}h"] = hi
            m[f"w{nm}l"] = lo
        m["woT"] = np.ascontiguousarray(
            (wo[:, esl].T.astype(np.float32) / np.float32(WS)).astype(np.float16)
        )
        m.update(consts)
        maps.append(m)
    return maps


def kernel(q, k, v, wq, wk, wv, wo):
    q, k, v = (np.asarray(a, dtype=np.float16) for a in (q, k, v))
    wq, wk, wv, wo = (np.asarray(a, dtype=np.float16) for a in (wq, wk, wv, wo))
    from concourse.bass_utils import run_bass_kernel_spmd

    nc = _get_nc()
    maps = _in_maps(q, k, v, wq, wk, wv, wo)
    res = run_bass_kernel_spmd(nc, maps, core_ids=list(range(N_CORES)))
    out = np.zeros((B, S, DIM), dtype=np.float32)
    for c in range(N_CORES):
        out[c // 4] += np.asarray(res.results[c]["y"]).astype(np.float32)
    return out.astype(np.float16)


# revision 9
# speedup vs baseline: 1.0215x; 1.0215x over previous
"""Trainium2 Bass kernel for sliding-window causal attention block.

Reference computation (per batch b):
  qh = (q @ wq.T)  -> [S, H, Dh], RoPE'd; kh likewise; vh = v @ wv.T
  scores = qh . kh / sqrt(Dh), sliding-window causal (j in (i-512, i])
  out = softmax(scores) @ vh  -> [S, H*Dh] @ wo.T -> [S, D]

Sharding: 8 cores = 2 batches x 4 head-groups (4 heads each).
Each core computes y_part[b] = attn(heads g) @ wo[:, g].T  (f16 partial);
host sums the 4 partials per batch and casts to f16.

Key optimizations over the f16 baseline (CoreSim makespan 126.6us):
  - Projections in compensated fp8 with DoubleRow perf mode (0.5 cyc/col):
    x = x_hi(e4m3) + x_lo(e5m2), w*32 = w_hi(e4m3) + w_lo(e5m2), both split
    on the host; on-chip x@w = xh@wh + xl@wh + xh@wl (lo x lo dropped).
    12 DoubleRow matmuls replace 8 f16 matmuls per K=1024 contraction:
    0.75x PE cost at ~0.2% rms error (measured rel-err 0.0016 vs 2e-2 gate).
    The *32 weight scale keeps w out of e4m3's subnormal range; it is
    folded out via exp scale 2^-13 (q,k) and woT/32 (v path).
  - Window masks: no PE mask transposes into PSUM; instead exp runs
    unmasked and the 1-2 boundary chunks of each (tile, head) are zeroed
    post-exp with in-place binary-mask multiplies on the idle GpSimd
    engine (PE -12us).
  - Phase interleaving: attention for q-tiles 4(sc-1)..4sc-1 is emitted
    between projection s-chunks sc and sc+1, so PE never drains while
    ACT/DVE catch up, and the input DMAs overlap compute.
  - DMA consolidation: each x tensor chunk loads with a single 3D-AP DMA
    ([128, 8, 512] from the [D, S] DRAM view) instead of 8 per-kc DMAs;
    y stores issue from the Pool queue to unload SP.
  - Engine balance (v1 cost model): PE ~72us, DVE ~56, ACT ~53, Pool ~38,
    SP ~47.  RoPE runs as ACT copy + PE perm-matmul + DVE t2-mul +
    Pool t1-mul + Pool add; v copies and y casts on DVE; q/k copies on ACT.
"""

import os
import sys

import numpy as np

for _p in ("/opt/trn_rl_repo", "/root/.axon_site/_ro/trn_rl_repo"):
    if os.path.isdir(_p) and _p not in sys.path:
        sys.path.insert(0, _p)

import ml_dtypes

DIM = 1024
NUM_HEADS = 16
HEAD_DIM = 64
WINDOW = 512
S = 2048
B = 2
HPC = 4  # heads per core
E = HPC * HEAD_DIM  # 256 = per-core hidden slice
N_CORES = 8
ST = S // 128  # 16 query tiles of 128
KC = DIM // 128  # 8 contraction chunks for projections
WS = 32.0  # weight pre-scale (keeps w out of e4m3 subnormals)
EXP_SCALE = 0.125 / (WS * WS)  # 2^-13, folds away q,k scales

E4 = ml_dtypes.float8_e4m3
E5 = ml_dtypes.float8_e5m2


def _rope_tables():
    # A/B factor tables in the RoPE-permuted [p, s] layout, f32.
    f = np.arange(32, dtype=np.float64)
    inv_freq = 1.0 / (10000.0 ** (2.0 * f / HEAD_DIM))  # [32]
    ang = np.arange(S, dtype=np.float64)[None, :] * inv_freq[:, None]  # [32, S]
    cos = np.cos(ang)
    sin = np.sin(ang)
    A = np.empty((128, S), dtype=np.float32)
    Bt = np.empty((128, S), dtype=np.float32)
    for blk in range(2):  # two 64-partition head blocks per tile
        o = blk * 64
        A[o : o + 32] = cos
        A[o + 32 : o + 64] = cos
        Bt[o : o + 32] = -sin
        Bt[o + 32 : o + 64] = sin
    return A, Bt


def _consts():
    A, Bt = _rope_tables()
    # binary masks in the transposed-scores [jj, ii] orientation
    jj = np.arange(128)[:, None]
    ii = np.arange(128)[None, :]
    mbinD = (jj <= ii).astype(np.float16)  # diagonal chunk: key j <= query i
    mbinL = (jj > ii).astype(np.float16)  # leftmost chunk: j > i - WINDOW
    permM = np.zeros((128, 128), dtype=np.float16)
    for m in range(128):
        partner = m + 32 if (m % 64) < 32 else m - 32
        permM[partner, m] = 1.0
    ident = np.eye(128, dtype=np.float16)
    return {
        "ropeA": A.astype(np.float16),
        "ropeB": Bt.astype(np.float16),
        "mbinD": mbinD,
        "mbinL": mbinL,
        "permM": permM,
        "ident": ident,
    }


def _head_perm():
    # within each head: evens then odds
    p = np.empty(E, dtype=np.int64)
    for h in range(HPC):
        base = h * HEAD_DIM
        p[base : base + 32] = base + np.arange(0, 64, 2)
        p[base + 32 : base + 64] = base + np.arange(1, 64, 2)
    return p


def _split8(x64):
    """hi = e4m3(x), lo = e5m2(x - hi); x64 must be float32/64."""
    hi = np.asarray(x64).astype(E4)
    lo = (np.asarray(x64, dtype=np.float32) - hi.astype(np.float32)).astype(E5)
    return hi, lo


def build_bass(do_compile=True):
    import concourse.bacc as bacc
    import concourse.mybir as mybir
    import concourse.tile as tile

    f16 = mybir.dt.float16
    f32 = mybir.dt.float32
    e4 = mybir.dt.float8e4
    e5 = mybir.dt.float8e5
    DR = mybir.MatmulPerfMode.DoubleRow
    Exp = mybir.ActivationFunctionType.Exp

    nc = bacc.Bacc("TRN2")

    xin = {}
    for t in ("q", "k", "v"):
        for p, dt in (("h", e4), ("l", e5)):
            xin[t + p] = nc.dram_tensor(f"x{t}{p}", [DIM, S], dt, kind="ExternalInput")
    win = {}
    for t in ("q", "k", "v"):
        for p, dt in (("h", e4), ("l", e5)):
            win[t + p] = nc.dram_tensor(f"w{t}{p}", [DIM, E], dt, kind="ExternalInput")
    woT = nc.dram_tensor("woT", [E, DIM], f16, kind="ExternalInput")
    ropeA = nc.dram_tensor("ropeA", [128, S], f16, kind="ExternalInput")
    ropeB = nc.dram_tensor("ropeB", [128, S], f16, kind="ExternalInput")
    mbinD = nc.dram_tensor("mbinD", [128, 128], f16, kind="ExternalInput")
    mbinL = nc.dram_tensor("mbinL", [128, 128], f16, kind="ExternalInput")
    permM = nc.dram_tensor("permM", [128, 128], f16, kind="ExternalInput")
    ident = nc.dram_tensor("ident", [128, 128], f16, kind="ExternalInput")
    y = nc.dram_tensor("y", [S, DIM], f16, kind="ExternalOutput")

    with tile.TileContext(nc) as tc:
        # All pools stay open for the whole kernel (no recycled-slot release
        # fan-in; see baseline notes on DVE sync-wait limits).
        with tc.tile_pool(name="res", bufs=1) as res, \
             tc.tile_pool(name="xp", bufs=2) as xp, \
             tc.tile_pool(name="rawp", bufs=2) as rawp, \
             tc.tile_pool(name="t12", bufs=2) as t12, \
             tc.tile_pool(name="ptp", bufs=4) as ptp, \
             tc.tile_pool(name="sb2", bufs=2) as sb2, \
             tc.tile_pool(name="yb", bufs=3) as yb, \
             tc.tile_pool(name="pu", bufs=3, space="PSUM") as pu, \
             tc.tile_pool(name="pst", bufs=2, space="PSUM") as stp, \
             tc.tile_pool(name="pop", bufs=1, space="PSUM") as op:
            # resident tensors
            qT = res.tile([128, 2, S], f16)
            kT = res.tile([128, 2, S], f16)
            v_sb = res.tile([128, ST, HPC, 65], f16)
            woT_sb = res.tile([128, 2, DIM], f16)
            mbinD_sb = res.tile([128, 128], f16)
            mbinL_sb = res.tile([128, 128], f16)
            ident_sb = res.tile([128, 128], f16)
            A_sb = res.tile([128, S], f16)
            B_sb = res.tile([128, S], f16)
            perm_sb = res.tile([128, 128], f16)
            w_sb = {}
            for t in ("q", "k", "v"):
                for p, dt in (("h", e4), ("l", e5)):
                    w_sb[t + p] = res.tile([128, KC, E], dt, name=f"w_{t}{p}")


            # Startup queue plan (everything ordered by first-need time):
            #   SP:   wqh, xq0h, perm, wql, xq0l, then all x loads for sc>=1
            #   ACT:  wkh, xk0h, wkl, xk0l   (one-time ~4.7us)
            #   Pool: A, B, wv, xv0, masks, ident, woT  (one-time ~10us)
            def wload(t, p, eng):
                eng.dma_start(
                    out=w_sb[t + p],
                    in_=win[t + p][:].rearrange("(c p) e -> p c e", p=128),
                )

            def xload(sc, t, p, eng):
                ssl = slice(sc * 512, (sc + 1) * 512)
                dt = e4 if p == "h" else e5
                xt = xp.tile([128, KC, 512], dt, tag=f"x{t}{p}", name=f"x{t}{p}_t")
                eng.dma_start(
                    out=xt,
                    in_=xin[t + p][:, ssl].rearrange("(c p) s -> p c s", p=128),
                )
                return xt

            xts = {}
            wload("q", "h", nc.sync)
            xts["qh"] = xload(0, "q", "h", nc.sync)
            wload("k", "h", nc.scalar)
            nc.gpsimd.dma_start(out=A_sb, in_=ropeA[:])
            nc.sync.dma_start(out=perm_sb, in_=permM[:])
            wload("q", "l", nc.sync)
            xts["ql"] = xload(0, "q", "l", nc.sync)
            xts["kh"] = xload(0, "k", "h", nc.scalar)
            wload("k", "l", nc.scalar)
            xts["kl"] = xload(0, "k", "l", nc.scalar)
            nc.gpsimd.dma_start(out=B_sb, in_=ropeB[:])
            wload("v", "h", nc.gpsimd)
            wload("v", "l", nc.gpsimd)
            xts["vh"] = xload(0, "v", "h", nc.gpsimd)
            xts["vl"] = xload(0, "v", "l", nc.gpsimd)
            nc.gpsimd.dma_start(out=mbinD_sb, in_=mbinD[:])
            nc.gpsimd.dma_start(out=mbinL_sb, in_=mbinL[:])
            nc.gpsimd.dma_start(out=ident_sb, in_=ident[:])
            nc.gpsimd.dma_start(
                out=woT_sb, in_=woT[:].rearrange("(c p) n -> p c n", p=128)
            )
            nc.any.memset(v_sb[:, :, :, 64:65], 1.0)

            def load_x(sc):
                out = {}
                for t in ("q", "k", "v"):
                    for p in ("h", "l"):
                        out[t + p] = xload(sc, t, p, nc.sync)
                return out

            def proj_dr(ps_out, lhs_h, lhs_l, rhs_h, rhs_l, lhsl, rhsl):
                # 12 DoubleRow matmuls: hi@hi + lo@hi + hi@lo over 4 kc-pairs
                first = True
                for wh, xh in ((lhs_h, rhs_h), (lhs_l, rhs_h), (lhs_h, rhs_l)):
                    for c in range(4):
                        cs = slice(2 * c, 2 * c + 2)
                        nc.tensor.matmul(
                            ps_out,
                            lhsT=wh[:, cs, lhsl],
                            rhs=xh[:, cs, rhsl],
                            start=first,
                            stop=(wh is lhs_h and xh is rhs_l and c == 3),
                            perf_mode=DR,
                        )
                        first = False

            def attn_tile(t):
                c0 = max(0, t - 4)
                ncv = t - c0 + 1
                tsl = slice(t * 128, (t + 1) * 128)
                # one PSUM bank holds the PV accumulator (floats 0:264) and
                # the two attnT transpose scratch regions (f16 bitcast views)
                pot = op.tile([128, 512], f32, tag="po")
                po = pot[:, 0:264].rearrange("p (h d) -> p h d", h=HPC)
                for h in range(HPC):
                    ec, hh = h // 2, h % 2
                    psl = slice(hh * 64, (hh + 1) * 64)
                    pst = stp.tile([128, 5, 128], f32, tag="st")
                    for si in range(ncv):
                        c = c0 + si
                        nc.tensor.matmul(
                            pst[:, si, :],
                            lhsT=kT[psl, ec, c * 128 : (c + 1) * 128],
                            rhs=qT[psl, ec, tsl],
                            start=True,
                            stop=True,
                        )
                    pt = ptp.tile([128, 5, 128], f16, tag="pt")
                    nc.scalar.activation(
                        pt[:, 0:ncv, :], pst[:, 0:ncv, :], Exp, scale=EXP_SCALE
                    )
                    # zero invalid boundary entries post-exp (GpSimd, SBUF-only)
                    if t >= 4:
                        nc.gpsimd.tensor_mul(pt[:, 0, :], pt[:, 0, :], mbinL_sb)
                    nc.gpsimd.tensor_mul(
                        pt[:, ncv - 1, :], pt[:, ncv - 1, :], mbinD_sb
                    )
                    for si in range(ncv):
                        c = c0 + si
                        nc.tensor.matmul(
                            po[:, h, 0:65],
                            lhsT=pt[:, si, :],
                            rhs=v_sb[:, c, h, :],
                            start=(si == 0),
                            stop=(si == ncv - 1),
                        )
                rc = sb2.tile([128, HPC, 1], f32, tag="rc")
                nc.vector.reciprocal(rc, po[:, :, 64:65])
                attn_t = sb2.tile([128, HPC, 64], f16, tag="attn")
                nc.vector.tensor_mul(
                    attn_t, po[:, :, 0:64], rc.broadcast_to([128, HPC, 64])
                )
                attnT_t = sb2.tile([128, 2, 128], f16, tag="attnT", bufs=3)
                attn_flat = attn_t.rearrange("p h d -> p (h d)")
                for ec in range(2):
                    ptr = pot[:, 264 + 64 * ec : 328 + 64 * ec].bitcast(f16)
                    nc.tensor.transpose(
                        ptr, attn_flat[:, ec * 128 : (ec + 1) * 128], ident_sb
                    )
                    nc.vector.tensor_copy(attnT_t[:, ec, :], ptr)
                return attnT_t

            def y_emit(t, attnT_t, tail=False):
                tsl = slice(t * 128, (t + 1) * 128)
                y_sb = yb.tile([128, DIM], f16, tag="ysb")
                for nch in range(2):
                    nsl = slice(nch * 512, (nch + 1) * 512)
                    py = pu.tile([128, 512], f32, tag="pu", name="py")
                    for ec in range(2):
                        nc.tensor.matmul(
                            py,
                            lhsT=attnT_t[:, ec, :],
                            rhs=woT_sb[:, ec, nsl],
                            start=(ec == 0),
                            stop=(ec == 1),
                        )
                    nc.vector.tensor_copy(y_sb[:, nsl], py)
                    if tail:
                        nc.sync.dma_start(out=y[tsl, nsl], in_=y_sb[:, nsl])
                if not tail:
                    nc.gpsimd.dma_start(out=y[tsl, :], in_=y_sb)

            pend = []

            def run_tile(t):
                attnT_t = attn_tile(t)
                pend.append((t, attnT_t))
                if len(pend) > 1:
                    y_emit(*pend.pop(0))

            def qk_unit(xts, sc, t, ec, out_sb, raw):
                ssl = slice(sc * 512, (sc + 1) * 512)
                esl = slice(ec * 128, (ec + 1) * 128)
                ps = pu.tile([128, 512], f32, tag="pu", name="ps_qk")
                proj_dr(
                    ps, w_sb[t + "h"], w_sb[t + "l"],
                    xts[t + "h"], xts[t + "l"], esl, slice(None),
                )
                nc.scalar.copy(raw[:, ec, :], ps)
                psh = pu.tile([128, 512], f32, tag="pu", name="psh")
                nc.tensor.matmul(
                    psh, lhsT=perm_sb, rhs=raw[:, ec, :],
                    start=True, stop=True,
                )
                t2 = t12.tile([128, 512], f16, tag="t2")
                nc.vector.tensor_mul(t2, psh, B_sb[:, ssl])
                t1 = t12.tile([128, 512], f16, tag="t1")
                nc.gpsimd.tensor_mul(t1, raw[:, ec, :], A_sb[:, ssl])
                nc.gpsimd.tensor_add(out_sb[:, ec, ssl], t1, t2)

            def v_unit(xts, sc, st4):
                sc16 = sc * 4 + st4
                stsl = slice(st4 * 128, (st4 + 1) * 128)
                ps = pu.tile([128, 512], f32, tag="pu", name="ps_v")
                proj_dr(
                    ps[:, 0:E], xts["vh"], xts["vl"],
                    w_sb["vh"], w_sb["vl"], stsl, slice(None),
                )
                nc.vector.tensor_copy(
                    v_sb[:, sc16, :, 0:64],
                    ps[:, 0:E].rearrange("p (h d) -> p h d", h=HPC),
                )

            def proj_units(xts, sc):
                raws = {
                    "q": rawp.tile([128, 2, 512], f16, tag="rawq", name="rawq"),
                    "k": rawp.tile([128, 2, 512], f16, tag="rawk", name="rawk"),
                }
                units = []
                for t, out_sb in (("q", qT), ("k", kT)):
                    for ec in range(2):
                        units.append(
                            lambda t=t, ec=ec, o=out_sb: qk_unit(
                                xts, sc, t, ec, o, raws[t]
                            )
                        )
                for st4 in range(4):
                    units.append(lambda st4=st4: v_unit(xts, sc, st4))
                return units

            for sc in range(4):
                xts_next = load_x(sc + 1) if sc + 1 < 4 else None
                units = proj_units(xts, sc)
                if sc == 0:
                    for u in units:
                        u()
                else:
                    # interleave: attn tile of the previous chunk between
                    # pairs of projection units so PE never drains
                    for i in range(4):
                        run_tile(4 * (sc - 1) + i)
                        units[2 * i]()
                        units[2 * i + 1]()
                xts = xts_next
            for t in range(12, ST):
                run_tile(t)
            while pend:
                t_, a_ = pend.pop(0)
                y_emit(t_, a_, tail=not pend)
    if do_compile:
        nc.compile()
    return nc


_CACHE = {}


def _get_nc():
    if "nc" not in _CACHE:
        _CACHE["nc"] = build_bass()
    return _CACHE["nc"]


def _in_maps(q, k, v, wq, wk, wv, wo):
    consts = _consts()
    perm = _head_perm()
    # per-batch x splits, shared by the 4 head-group cores of that batch
    xsplit = []
    for b in range(B):
        m = {}
        for nm, x in (("q", q), ("k", k), ("v", v)):
            hi, lo = _split8(np.ascontiguousarray(x[b].T, dtype=np.float32))
            m[f"x{nm}h"] = hi
            m[f"x{nm}l"] = lo
        xsplit.append(m)
    maps = []
    for c in range(N_CORES):
        b, g = c // 4, c % 4
        esl = slice(g * E, (g + 1) * E)
        m = dict(xsplit[b])
        for nm, w, permute in (("q", wq, True), ("k", wk, True), ("v", wv, False)):
            w_c = w[esl]
            if permute:
                w_c = w_c[perm]
            wt = np.ascontiguousarray(w_c.T, dtype=np.float32) * np.float32(WS)
            hi, lo = _split8(wt)
            m[f"w{nm}h"] = hi
            m[f"w{nm}l"] = lo
        m["woT"] = np.ascontiguousarray(
            (wo[:, esl].T.astype(np.float32) / np.float32(WS)).astype(np.float16)
        )
        m.update(consts)
        maps.append(m)
    return maps


def kernel(q, k, v, wq, wk, wv, wo):
    q, k, v = (np.asarray(a, dtype=np.float16) for a in (q, k, v))
    wq, wk, wv, wo = (np.asarray(a, dtype=np.float16) for a in (wq, wk, wv, wo))
    from concourse.bass_utils import run_bass_kernel_spmd

    nc = _get_nc()
    maps = _in_maps(q, k, v, wq, wk, wv, wo)
    res = run_bass_kernel_spmd(nc, maps, core_ids=list(range(N_CORES)))
    out = np.zeros((B, S, DIM), dtype=np.float32)
    for c in range(N_CORES):
        out[c // 4] += np.asarray(res.results[c]["y"]).astype(np.float32)
    return out.astype(np.float16)


# revision 10
# speedup vs baseline: 1.0330x; 1.0113x over previous
"""Trainium2 Bass kernel for sliding-window causal attention block.

Reference computation (per batch b):
  qh = (q @ wq.T)  -> [S, H, Dh], RoPE'd; kh likewise; vh = v @ wv.T
  scores = qh . kh / sqrt(Dh), sliding-window causal (j in (i-512, i])
  out = softmax(scores) @ vh  -> [S, H*Dh] @ wo.T -> [S, D]

Sharding: 8 cores = 2 batches x 4 head-groups (4 heads each).
Each core computes y_part[b] = attn(heads g) @ wo[:, g].T  (f16 partial);
host sums the 4 partials per batch and casts to f16.

Key optimizations over the f16 baseline (CoreSim makespan 126.6us):
  - Projections in compensated fp8 with DoubleRow perf mode (0.5 cyc/col):
    x = x_hi(e4m3) + x_lo(e5m2), w*32 = w_hi(e4m3) + w_lo(e5m2), both split
    on the host; on-chip x@w = xh@wh + xl@wh + xh@wl (lo x lo dropped).
    12 DoubleRow matmuls replace 8 f16 matmuls per K=1024 contraction:
    0.75x PE cost at ~0.2% rms error (measured rel-err 0.0016 vs 2e-2 gate).
    The *32 weight scale keeps w out of e4m3's subnormal range; it is
    folded out via exp scale 2^-13 (q,k) and woT/32 (v path).
  - Window masks: no PE mask transposes into PSUM; instead exp runs
    unmasked and the 1-2 boundary chunks of each (tile, head) are zeroed
    post-exp with in-place binary-mask multiplies on the idle GpSimd
    engine (PE -12us).
  - Phase interleaving: attention for q-tiles 4(sc-1)..4sc-1 is emitted
    between projection s-chunks sc and sc+1, so PE never drains while
    ACT/DVE catch up, and the input DMAs overlap compute.
  - DMA consolidation: each x tensor chunk loads with a single 3D-AP DMA
    ([128, 8, 512] from the [D, S] DRAM view) instead of 8 per-kc DMAs;
    y stores issue from the Pool queue to unload SP.
  - Engine balance (v1 cost model): PE ~72us, DVE ~56, ACT ~53, Pool ~38,
    SP ~47.  RoPE runs as ACT copy + PE perm-matmul + DVE t2-mul +
    Pool t1-mul + Pool add; v copies and y casts on DVE; q/k copies on ACT.
"""

import os
import sys

import numpy as np

for _p in ("/opt/trn_rl_repo", "/root/.axon_site/_ro/trn_rl_repo"):
    if os.path.isdir(_p) and _p not in sys.path:
        sys.path.insert(0, _p)

import ml_dtypes

DIM = 1024
NUM_HEADS = 16
HEAD_DIM = 64
WINDOW = 512
S = 2048
B = 2
HPC = 4  # heads per core
E = HPC * HEAD_DIM  # 256 = per-core hidden slice
N_CORES = 8
ST = S // 128  # 16 query tiles of 128
KC = DIM // 128  # 8 contraction chunks for projections
WS = 32.0  # weight pre-scale (keeps w out of e4m3 subnormals)
EXP_SCALE = 0.125 / (WS * WS)  # 2^-13, folds away q,k scales

E4 = ml_dtypes.float8_e4m3
E5 = ml_dtypes.float8_e5m2


def _rope_tables():
    # A/B factor tables in the RoPE-permuted [p, s] layout, f32.
    f = np.arange(32, dtype=np.float64)
    inv_freq = 1.0 / (10000.0 ** (2.0 * f / HEAD_DIM))  # [32]
    ang = np.arange(S, dtype=np.float64)[None, :] * inv_freq[:, None]  # [32, S]
    cos = np.cos(ang)
    sin = np.sin(ang)
    A = np.empty((128, S), dtype=np.float32)
    Bt = np.empty((128, S), dtype=np.float32)
    for blk in range(2):  # two 64-partition head blocks per tile
        o = blk * 64
        A[o : o + 32] = cos
        A[o + 32 : o + 64] = cos
        Bt[o : o + 32] = -sin
        Bt[o + 32 : o + 64] = sin
    return A, Bt


def _consts():
    A, Bt = _rope_tables()
    # binary masks in the transposed-scores [jj, ii] orientation
    jj = np.arange(128)[:, None]
    ii = np.arange(128)[None, :]
    mbinD = (jj <= ii).astype(np.float16)  # diagonal chunk: key j <= query i
    mbinL = (jj > ii).astype(np.float16)  # leftmost chunk: j > i - WINDOW
    permM = np.zeros((128, 128), dtype=np.float16)
    for m in range(128):
        partner = m + 32 if (m % 64) < 32 else m - 32
        permM[partner, m] = 1.0
    ident = np.eye(128, dtype=np.float16)
    return {
        "ropeA": A.astype(np.float16),
        "ropeB": Bt.astype(np.float16),
        "mbinD": mbinD,
        "mbinL": mbinL,
        "permM": permM,
        "ident": ident,
    }


def _head_perm():
    # within each head: evens then odds
    p = np.empty(E, dtype=np.int64)
    for h in range(HPC):
        base = h * HEAD_DIM
        p[base : base + 32] = base + np.arange(0, 64, 2)
        p[base + 32 : base + 64] = base + np.arange(1, 64, 2)
    return p


def _split8(x64):
    """hi = e4m3(x), lo = e5m2(x - hi); x64 must be float32/64."""
    hi = np.asarray(x64).astype(E4)
    lo = (np.asarray(x64, dtype=np.float32) - hi.astype(np.float32)).astype(E5)
    return hi, lo


def build_bass(do_compile=True):
    import concourse.bacc as bacc
    import concourse.mybir as mybir
    import concourse.tile as tile

    f16 = mybir.dt.float16
    f32 = mybir.dt.float32
    e4 = mybir.dt.float8e4
    e5 = mybir.dt.float8e5
    DR = mybir.MatmulPerfMode.DoubleRow
    Exp = mybir.ActivationFunctionType.Exp

    nc = bacc.Bacc("TRN2")

    xin = {}
    for t in ("q", "k", "v"):
        for p, dt in (("h", e4), ("l", e5)):
            xin[t + p] = nc.dram_tensor(f"x{t}{p}", [DIM, S], dt, kind="ExternalInput")
    win = {}
    for t in ("q", "k", "v"):
        for p, dt in (("h", e4), ("l", e5)):
            win[t + p] = nc.dram_tensor(f"w{t}{p}", [DIM, E], dt, kind="ExternalInput")
    woT = nc.dram_tensor("woT", [E, DIM], f16, kind="ExternalInput")
    ropeA = nc.dram_tensor("ropeA", [128, S], f16, kind="ExternalInput")
    ropeB = nc.dram_tensor("ropeB", [128, S], f16, kind="ExternalInput")
    mbinD = nc.dram_tensor("mbinD", [128, 128], f16, kind="ExternalInput")
    mbinL = nc.dram_tensor("mbinL", [128, 128], f16, kind="ExternalInput")
    permM = nc.dram_tensor("permM", [128, 128], f16, kind="ExternalInput")
    ident = nc.dram_tensor("ident", [128, 128], f16, kind="ExternalInput")
    y = nc.dram_tensor("y", [S, DIM], f16, kind="ExternalOutput")

    with tile.TileContext(nc) as tc:
        # All pools stay open for the whole kernel (no recycled-slot release
        # fan-in; see baseline notes on DVE sync-wait limits).
        with tc.tile_pool(name="res", bufs=1) as res, \
             tc.tile_pool(name="xp", bufs=2) as xp, \
             tc.tile_pool(name="rawp", bufs=2) as rawp, \
             tc.tile_pool(name="t12", bufs=2) as t12, \
             tc.tile_pool(name="ptp", bufs=4) as ptp, \
             tc.tile_pool(name="sb2", bufs=2) as sb2, \
             tc.tile_pool(name="yb", bufs=3) as yb, \
             tc.tile_pool(name="pu", bufs=3, space="PSUM") as pu, \
             tc.tile_pool(name="pst", bufs=2, space="PSUM") as stp, \
             tc.tile_pool(name="pop", bufs=1, space="PSUM") as op:
            # resident tensors
            qT = res.tile([128, 2, S], f16)
            kT = res.tile([128, 2, S], f16)
            v_sb = res.tile([128, ST, HPC, 65], f16)
            woT_sb = res.tile([128, 2, DIM], f16)
            mbinD_sb = res.tile([128, 128], f16)
            mbinL_sb = res.tile([128, 128], f16)
            ident_sb = res.tile([128, 128], f16)
            A_sb = res.tile([128, S], f16)
            B_sb = res.tile([128, S], f16)
            perm_sb = res.tile([128, 128], f16)
            w_sb = {}
            for t in ("q", "k", "v"):
                for p, dt in (("h", e4), ("l", e5)):
                    w_sb[t + p] = res.tile([128, KC, E], dt, name=f"w_{t}{p}")


            # Startup queue plan (everything ordered by first-need time):
            #   SP:   wqh, xq0h, perm, wql, xq0l, then all x loads for sc>=1
            #   ACT:  wkh, xk0h, wkl, xk0l   (one-time ~4.7us)
            #   Pool: A, B, wv, xv0, masks, ident, woT  (one-time ~10us)
            def wload(t, p, eng):
                eng.dma_start(
                    out=w_sb[t + p],
                    in_=win[t + p][:].rearrange("(c p) e -> p c e", p=128),
                )

            def xload(sc, t, p, eng):
                ssl = slice(sc * 512, (sc + 1) * 512)
                dt = e4 if p == "h" else e5
                xt = xp.tile([128, KC, 512], dt, tag=f"x{t}{p}", name=f"x{t}{p}_t")
                eng.dma_start(
                    out=xt,
                    in_=xin[t + p][:, ssl].rearrange("(c p) s -> p c s", p=128),
                )
                return xt

            def xload_half(xt, sc, t, p, half, eng):
                # load kc chunks [4*half, 4*half+4) of an x tile so the first
                # DoubleRow matmuls can start before the full tile lands
                ssl = slice(sc * 512, (sc + 1) * 512)
                cs = slice(half * 4, half * 4 + 4)
                csl = slice(half * 512, half * 512 + 512)
                eng.dma_start(
                    out=xt[:, cs, :],
                    in_=xin[t + p][csl, ssl].rearrange("(c p) s -> p c s", p=128),
                )

            xts = {}
            for t in ("q", "k"):
                for p, dt in (("h", e4), ("l", e5)):
                    xts[t + p] = xp.tile(
                        [128, KC, 512], dt, tag=f"x{t}{p}", name=f"x{t}{p}_t"
                    )
            wload("q", "h", nc.scalar)
            xload_half(xts["qh"], 0, "q", "h", 0, nc.sync)
            wload("q", "l", nc.scalar)
            xload_half(xts["ql"], 0, "q", "l", 0, nc.sync)
            nc.scalar.dma_start(out=perm_sb, in_=permM[:])
            xload_half(xts["qh"], 0, "q", "h", 1, nc.sync)
            xload_half(xts["ql"], 0, "q", "l", 1, nc.sync)
            wload("k", "h", nc.scalar)
            wload("k", "l", nc.scalar)
            nc.gpsimd.dma_start(out=A_sb, in_=ropeA[:])
            xload_half(xts["kh"], 0, "k", "h", 0, nc.sync)
            xload_half(xts["kl"], 0, "k", "l", 0, nc.sync)
            xload_half(xts["kh"], 0, "k", "h", 1, nc.sync)
            xload_half(xts["kl"], 0, "k", "l", 1, nc.sync)
            nc.gpsimd.dma_start(out=B_sb, in_=ropeB[:])
            wload("v", "h", nc.gpsimd)
            wload("v", "l", nc.gpsimd)
            xts["vh"] = xload(0, "v", "h", nc.gpsimd)
            xts["vl"] = xload(0, "v", "l", nc.gpsimd)
            nc.gpsimd.dma_start(out=mbinD_sb, in_=mbinD[:])
            nc.gpsimd.dma_start(out=mbinL_sb, in_=mbinL[:])
            nc.gpsimd.dma_start(out=ident_sb, in_=ident[:])
            nc.gpsimd.dma_start(
                out=woT_sb, in_=woT[:].rearrange("(c p) n -> p c n", p=128)
            )
            nc.any.memset(v_sb[:, :, :, 64:65], 1.0)

            def load_x(sc):
                out = {}
                for t in ("q", "k", "v"):
                    for p in ("h", "l"):
                        out[t + p] = xload(sc, t, p, nc.sync)
                return out

            def proj_dr(ps_out, lhs_h, lhs_l, rhs_h, rhs_l, lhsl, rhsl):
                # 12 DoubleRow matmuls: hi@hi + lo@hi + hi@lo over 4 kc-pairs
                first = True
                for wh, xh in ((lhs_h, rhs_h), (lhs_l, rhs_h), (lhs_h, rhs_l)):
                    for c in range(4):
                        cs = slice(2 * c, 2 * c + 2)
                        nc.tensor.matmul(
                            ps_out,
                            lhsT=wh[:, cs, lhsl],
                            rhs=xh[:, cs, rhsl],
                            start=first,
                            stop=(wh is lhs_h and xh is rhs_l and c == 3),
                            perf_mode=DR,
                        )
                        first = False

            def attn_tile(t):
                c0 = max(0, t - 4)
                ncv = t - c0 + 1
                tsl = slice(t * 128, (t + 1) * 128)
                # one PSUM bank holds the PV accumulator (floats 0:264) and
                # the two attnT transpose scratch regions (f16 bitcast views)
                pot = op.tile([128, 512], f32, tag="po")
                po = pot[:, 0:264].rearrange("p (h d) -> p h d", h=HPC)
                for h in range(HPC):
                    ec, hh = h // 2, h % 2
                    psl = slice(hh * 64, (hh + 1) * 64)
                    pst = stp.tile([128, 5, 128], f32, tag="st")
                    for si in range(ncv):
                        c = c0 + si
                        nc.tensor.matmul(
                            pst[:, si, :],
                            lhsT=kT[psl, ec, c * 128 : (c + 1) * 128],
                            rhs=qT[psl, ec, tsl],
                            start=True,
                            stop=True,
                        )
                    pt = ptp.tile([128, 5, 128], f16, tag="pt")
                    nc.scalar.activation(
                        pt[:, 0:ncv, :], pst[:, 0:ncv, :], Exp, scale=EXP_SCALE
                    )
                    # zero invalid boundary entries post-exp (GpSimd, SBUF-only)
                    if t >= 4:
                        nc.gpsimd.tensor_mul(pt[:, 0, :], pt[:, 0, :], mbinL_sb)
                    nc.gpsimd.tensor_mul(
                        pt[:, ncv - 1, :], pt[:, ncv - 1, :], mbinD_sb
                    )
                    for si in range(ncv):
                        c = c0 + si
                        nc.tensor.matmul(
                            po[:, h, 0:65],
                            lhsT=pt[:, si, :],
                            rhs=v_sb[:, c, h, :],
                            start=(si == 0),
                            stop=(si == ncv - 1),
                        )
                rc = sb2.tile([128, HPC, 1], f32, tag="rc")
                nc.vector.reciprocal(rc, po[:, :, 64:65])
                attn_t = sb2.tile([128, HPC, 64], f16, tag="attn")
                nc.vector.tensor_mul(
                    attn_t, po[:, :, 0:64], rc.broadcast_to([128, HPC, 64])
                )
                attnT_t = sb2.tile([128, 2, 128], f16, tag="attnT", bufs=3)
                attn_flat = attn_t.rearrange("p h d -> p (h d)")
                for ec in range(2):
                    ptr = pot[:, 264 + 64 * ec : 328 + 64 * ec].bitcast(f16)
                    nc.tensor.transpose(
                        ptr, attn_flat[:, ec * 128 : (ec + 1) * 128], ident_sb
                    )
                    nc.vector.tensor_copy(attnT_t[:, ec, :], ptr)
                return attnT_t

            def y_emit(t, attnT_t, tail=False):
                tsl = slice(t * 128, (t + 1) * 128)
                y_sb = yb.tile([128, DIM], f16, tag="ysb")
                for nch in range(2):
                    nsl = slice(nch * 512, (nch + 1) * 512)
                    py = pu.tile([128, 512], f32, tag="pu", name="py")
                    for ec in range(2):
                        nc.tensor.matmul(
                            py,
                            lhsT=attnT_t[:, ec, :],
                            rhs=woT_sb[:, ec, nsl],
                            start=(ec == 0),
                            stop=(ec == 1),
                        )
                    nc.vector.tensor_copy(y_sb[:, nsl], py)
                    if tail:
                        nc.sync.dma_start(out=y[tsl, nsl], in_=y_sb[:, nsl])
                if not tail:
                    nc.gpsimd.dma_start(out=y[tsl, :], in_=y_sb)

            pend = []

            def run_tile(t):
                attnT_t = attn_tile(t)
                pend.append((t, attnT_t))
                if len(pend) > 1:
                    y_emit(*pend.pop(0))

            def qk_unit(xts, sc, t, ec, out_sb, raw):
                ssl = slice(sc * 512, (sc + 1) * 512)
                esl = slice(ec * 128, (ec + 1) * 128)
                ps = pu.tile([128, 512], f32, tag="pu", name="ps_qk")
                proj_dr(
                    ps, w_sb[t + "h"], w_sb[t + "l"],
                    xts[t + "h"], xts[t + "l"], esl, slice(None),
                )
                nc.scalar.copy(raw[:, ec, :], ps)
                psh = pu.tile([128, 512], f32, tag="pu", name="psh")
                nc.tensor.matmul(
                    psh, lhsT=perm_sb, rhs=raw[:, ec, :],
                    start=True, stop=True,
                )
                t2 = t12.tile([128, 512], f16, tag="t2")
                nc.vector.tensor_mul(t2, psh, B_sb[:, ssl])
                t1 = t12.tile([128, 512], f16, tag="t1")
                nc.gpsimd.tensor_mul(t1, raw[:, ec, :], A_sb[:, ssl])
                nc.gpsimd.tensor_add(out_sb[:, ec, ssl], t1, t2)

            def v_unit(xts, sc, st4):
                sc16 = sc * 4 + st4
                stsl = slice(st4 * 128, (st4 + 1) * 128)
                ps = pu.tile([128, 512], f32, tag="pu", name="ps_v")
                proj_dr(
                    ps[:, 0:E], xts["vh"], xts["vl"],
                    w_sb["vh"], w_sb["vl"], stsl, slice(None),
                )
                nc.vector.tensor_copy(
                    v_sb[:, sc16, :, 0:64],
                    ps[:, 0:E].rearrange("p (h d) -> p h d", h=HPC),
                )

            def proj_units(xts, sc):
                raws = {
                    "q": rawp.tile([128, 2, 512], f16, tag="rawq", name="rawq"),
                    "k": rawp.tile([128, 2, 512], f16, tag="rawk", name="rawk"),
                }
                units = []
                for t, out_sb in (("q", qT), ("k", kT)):
                    for ec in range(2):
                        units.append(
                            lambda t=t, ec=ec, o=out_sb: qk_unit(
                                xts, sc, t, ec, o, raws[t]
                            )
                        )
                for st4 in range(4):
                    units.append(lambda st4=st4: v_unit(xts, sc, st4))
                return units

            for sc in range(4):
                xts_next = load_x(sc + 1) if sc + 1 < 4 else None
                units = proj_units(xts, sc)
                if sc == 0:
                    for u in units:
                        u()
                else:
                    # interleave: attn tile of the previous chunk between
                    # pairs of projection units so PE never drains
                    for i in range(4):
                        run_tile(4 * (sc - 1) + i)
                        units[2 * i]()
                        units[2 * i + 1]()
                xts = xts_next
            for t in range(12, ST):
                run_tile(t)
            while pend:
                t_, a_ = pend.pop(0)
                y_emit(t_, a_, tail=not pend)
    if do_compile:
        nc.compile()
    return nc


_CACHE = {}


def _get_nc():
    if "nc" not in _CACHE:
        _CACHE["nc"] = build_bass()
    return _CACHE["nc"]


def _in_maps(q, k, v, wq, wk, wv, wo):
    consts = _consts()
    perm = _head_perm()
    # per-batch x splits, shared by the 4 head-group cores of that batch
    xsplit = []
    for b in range(B):
        m = {}
        for nm, x in (("q", q), ("k", k), ("v", v)):
            hi, lo = _split8(np.ascontiguousarray(x[b].T, dtype=np.float32))
            m[f"x{nm}h"] = hi
            m[f"x{nm}l"] = lo
        xsplit.append(m)
    maps = []
    for c in range(N_CORES):
        b, g = c // 4, c % 4
        esl = slice(g * E, (g + 1) * E)
        m = dict(xsplit[b])
        for nm, w, permute in (("q", wq, True), ("k", wk, True), ("v", wv, False)):
            w_c = w[esl]
            if permute:
                w_c = w_c[perm]
            wt = np.ascontiguousarray(w_c.T, dtype=np.float32) * np.float32(WS)
            hi, lo = _split8(wt)
            m[f"w{nm}h"] = hi
            m[f"w{nm}l"] = lo
        m["woT"] = np.ascontiguousarray(
            (wo[:, esl].T.astype(np.float32) / np.float32(WS)).astype(np.float16)
        )
        m.update(consts)
        maps.append(m)
    return maps


def kernel(q, k, v, wq, wk, wv, wo):
    q, k, v = (np.asarray(a, dtype=np.float16) for a in (q, k, v))
    wq, wk, wv, wo = (np.asarray(a, dtype=np.float16) for a in (wq, wk, wv, wo))
    from concourse.bass_utils import run_bass_kernel_spmd

    nc = _get_nc()
    maps = _in_maps(q, k, v, wq, wk, wv, wo)
    res = run_bass_kernel_spmd(nc, maps, core_ids=list(range(N_CORES)))
    out = np.zeros((B, S, DIM), dtype=np.float32)
    for c in range(N_CORES):
        out[c // 4] += np.asarray(res.results[c]["y"]).astype(np.float32)
    return out.astype(np.float16)
